# revision 17
# baseline (speedup 1.0000x reference)
"""Trainium2 Bass kernel for the 2-layer cross-attention module.

Sharding: data-parallel over batch B=16 -> 2 batch elements per core, 8 cores,
no collectives. Per core the algebra is restructured by linearity so the two
giant [T,NZ]@[NZ,NZ] projections disappear:

  scores[t,h] = sum_i keys[t,i] * qk[i,h],   qk[i,h] = sum_{d in head h} Wk[i,hd] q[hd]
  vals[hd]    = sum_i wvals[h,i] * Wv[i,hd], wvals[h,i] = sum_t p[t,h] values[t,i]

All matmul inputs are bf16 (f32 PSUM accumulation); softmax/layernorm/attw run
in f32. The small linear chain (query_net / pred / attn_out / out) runs in a
feature-on-partitions column layout [128, 2] per tile.
"""

import numpy as np
import ml_dtypes

import concourse.mybir as mybir
from concourse import bacc, tile
from concourse.bass_utils import run_bass_kernel_spmd
from concourse.masks import make_identity

F32 = mybir.dt.float32
BF16 = mybir.dt.bfloat16
ALU = mybir.AluOpType
AXX = mybir.AxisListType
AF = mybir.ActivationFunctionType

B, T, NZ, H, DK = 16, 2048, 1024, 8, 128
NCORES, BP = 8, 2
NT, NI = T // 128, NZ // 128  # 16, 8
SCALE = float(1.0 / np.sqrt(DK))  # * attention_temperature (1.0)
EPS = 1e-5
N_LAYERS = 2
LRELU = 0.2

LAST_RESULT = {}  # test harness introspection: exec_time_ns etc.


def _np(x):
    return np.asarray(x)


def _bf(x):
    return np.ascontiguousarray(np.asarray(x, dtype=np.float32).astype(ml_dtypes.bfloat16))


def _f32(x):
    return np.ascontiguousarray(np.asarray(x), dtype=np.float32)


def _host_prep(values, keys, query_input, start_ind, end_ind, params):
    """Marshal full inputs into per-core shard dicts + build config."""
    values = _np(values)
    keys = _np(keys)
    query_input = _np(query_input)
    start_ind = _f32(start_ind)
    end_ind = _f32(end_ind)

    # ---- weights: flatten into named f32/bf16 arrays -------------------
    # linear spec: name -> (W [din,dout] bf16, bias [dout] f32, nk)
    lin_w = {}
    lin_b = {}

    def add_lin(name, p):
        W, b = p
        W = _f32(W)
        lin_w[name] = _bf(W)
        lin_b[name] = _f32(b).reshape(1, -1)

    qn = params["query_net"]
    for i, p in enumerate(qn):
        add_lin(f"qn{i}", p)
    for l, lp in enumerate(params["layers"]):
        add_lin(f"wq{l}", lp["q"])
        add_lin(f"wv{l}", lp["v"])
        add_lin(f"wao{l}", lp["attn_out"])
        for i, p in enumerate(lp["pred"]):
            add_lin(f"pred{l}_{i}", p)
        # k projection: host-transpose W, bias handled via scb path
        Wk, bk = lp["k"]
        lin_w[f"wkT{l}"] = _bf(_f32(Wk).T)
        lin_b[f"wkT{l}"] = np.zeros((1, NZ), np.float32)  # unused
        lin_b[f"bk{l}"] = _f32(bk).reshape(1, -1)
    add_lin("wout", params["out"])

    bias_nonzero = {k: bool(np.any(v)) for k, v in lin_b.items()}

    # ---- mask ----------------------------------------------------------
    tt = np.arange(T, dtype=np.float32)
    mask = (tt[None, :] < np.floor(start_ind)[:, None]) | (
        tt[None, :] > np.ceil(end_ind)[:, None]
    )  # [B, T], True = masked out
    fully = mask.all(axis=1)
    maskadd = np.where(mask, np.float32(-1e30), np.float32(0.0)).astype(np.float32)
    maskadd[fully] = 0.0  # fully-masked row -> uniform softmax (matches reference)
    mask_nontrivial = bool(maskadd.any())

    cfg = {
        "bias_nonzero": bias_nonzero,
        "mask": mask_nontrivial,
    }

    # ---- per-core shards ----------------------------------------------
    in_maps = []
    for c in range(NCORES):
        sl = slice(BP * c, BP * (c + 1))
        m = {
            "keysT": _bf(keys[sl].transpose(0, 2, 1)),          # [BP, NZ, T]
            "values": _bf(values[sl]),                          # [BP, T, NZ]
            # col layout [128, 16, BP]: partition p, ktile k, batch b
            "qin": _bf(query_input[sl].T.reshape(16, 128, BP).transpose(1, 0, 2)),
        }
        for k, v in lin_w.items():
            m[k] = v
        for k, v in lin_b.items():
            if bias_nonzero[k]:
                if k.startswith("bk"):
                    # column pack [128, 8]: partition d, col h -> bk[h*128+d]
                    m[k + "_col"] = v.reshape(H, DK).T.copy()
                else:
                    m[k + "_bias"] = v
        if mask_nontrivial:
            # expand per (b, h, t): same row for all h
            m["maskadd"] = np.repeat(maskadd[sl][:, None, :], H, axis=1).copy()
        in_maps.append(m)
    return in_maps, cfg


def build(cfg):
    bias_nz = cfg["bias_nonzero"]
    nc = bacc.Bacc("TRN2", target_bir_lowering=False, debug=False)

    keysT_d = nc.dram_tensor("keysT", [BP, NZ, T], BF16, kind="ExternalInput")
    values_d = nc.dram_tensor("values", [BP, T, NZ], BF16, kind="ExternalInput")
    qin_d = nc.dram_tensor("qin", [128, 16, BP], BF16, kind="ExternalInput")

    w_d = {}
    b_d = {}

    def decl_lin(name, din):
        w_d[name] = nc.dram_tensor(name, [din, NZ], BF16, kind="ExternalInput")
        if bias_nz.get(name, False):
            b_d[name] = nc.dram_tensor(name + "_bias", [1, NZ], F32, kind="ExternalInput")

    decl_lin("qn0", 2 * NZ)
    for i in range(1, 5):
        decl_lin(f"qn{i}", NZ)
    for l in range(N_LAYERS):
        decl_lin(f"wq{l}", NZ)
        decl_lin(f"wv{l}", NZ)
        decl_lin(f"wao{l}", NZ)
        for i in range(4):
            decl_lin(f"pred{l}_{i}", NZ)
        w_d[f"wkT{l}"] = nc.dram_tensor(f"wkT{l}", [NZ, NZ], BF16, kind="ExternalInput")
        if bias_nz.get(f"bk{l}", False):
            b_d[f"bk{l}"] = nc.dram_tensor(f"bk{l}_col", [128, H], F32, kind="ExternalInput")
    decl_lin("wout", NZ)
    if cfg["mask"]:
        mask_d = nc.dram_tensor("maskadd", [BP, H, T], F32, kind="ExternalInput")

    out_d = nc.dram_tensor("out", [BP, NZ], F32, kind="ExternalOutput")
    attw_d = nc.dram_tensor("attw", [BP, T], F32, kind="ExternalOutput")

    from contextlib import ExitStack

    with tile.TileContext(nc) as tc, ExitStack() as ctx:
        singles = ctx.enter_context(tc.tile_pool(name="singles", bufs=1))
        wpool = ctx.enter_context(tc.tile_pool(name="wpool", bufs=2))
        apool = ctx.enter_context(tc.tile_pool(name="apool", bufs=64))
        ppool = ctx.enter_context(tc.tile_pool(name="ppool", bufs=2))
        mpool = ctx.enter_context(tc.tile_pool(name="mpool", bufs=3))
        ps_mm2 = ctx.enter_context(tc.tile_pool(name="ps_mm2", bufs=3, space="PSUM"))
        ps_wide = ctx.enter_context(tc.tile_pool(name="ps_wide", bufs=2, space="PSUM"))
        ps_tp = ctx.enter_context(tc.tile_pool(name="ps_tp", bufs=2, space="PSUM"))

        # ---- constants -------------------------------------------------
        id_f = singles.tile([128, 128], F32, tag="id_f")
        make_identity(nc, id_f)
        id_b = singles.tile([128, 128], BF16, tag="id_b")
        make_identity(nc, id_b)
        ones_col = singles.tile([128, 1], F32, tag="ones_col")
        nc.vector.memset(ones_col, 1.0)
        ones_row = singles.tile([1, 128], F32, tag="ones_row")
        nc.vector.memset(ones_row, 1.0)
        ones2 = singles.tile([1, 2], F32, tag="ones2")
        nc.vector.memset(ones2, 1.0)
        sel8 = singles.tile([8, 1], F32, tag="sel8")
        nc.vector.memset(sel8, 1.0 / H)
        eps_t = singles.tile([1, 1], F32, tag="eps")
        nc.vector.memset(eps_t, EPS)

        # ---- persistent inputs (DMAs deferred: weights go first) -------
        # 3-D tiles: keysT_sb3[b] [128, NI, T]; values_sb3[b] [128, NT, NZ]
        keysT_sb3 = [singles.tile([128, NI, T], BF16, tag=f"kT{b}", name=f"kT{b}")
                     for b in range(BP)]
        values_sb3 = [singles.tile([128, NT, NZ], BF16, tag=f"v{b}", name=f"v{b}")
                      for b in range(BP)]
        keysT_sb = [[keysT_sb3[b][:, it, :] for it in range(NI)] for b in range(BP)]
        values_sb = [[values_sb3[b][:, tt, :] for tt in range(NT)] for b in range(BP)]

        def load_keysT(b):
            nc.sync.dma_start(out=keysT_sb3[b],
                              in_=keysT_d[b].rearrange("(it p) t -> p it t", p=128))

        def load_values(b):
            nc.sync.dma_start(out=values_sb3[b],
                              in_=values_d[b].rearrange("(tt p) i -> p tt i", p=128))

        qin_sb = singles.tile([128, 16, BP], BF16, tag="qin")
        nc.sync.dma_start(out=qin_sb, in_=qin_d[:, :, :])
        if cfg["mask"]:
            mask_sb = [singles.tile([H, T], F32, tag=f"mask{b}", name=f"mask{b}") for b in range(BP)]
            for b in range(BP):
                nc.sync.dma_start(out=mask_sb[b], in_=mask_d[b, :, :])

        # ---- helpers ---------------------------------------------------
        def load_w(name, nk):
            """Returns a lambda kt -> [128, 1024] AP; one batched DMA per 8 ktiles."""
            segs = []
            for s0 in range(0, nk, 8):
                w = wpool.tile([128, 8, NZ], BF16, tag="w", name=f"w_{name}_{s0}")
                nc.sync.dma_start(
                    out=w,
                    in_=w_d[name][s0 * 128:(s0 + 8) * 128, :].rearrange(
                        "(k p) n -> p k n", p=128))
                segs.append(w)
            return lambda kt: segs[kt // 8][:, kt % 8, :]

        def load_brow(name):
            if not bias_nz.get(name, False):
                return None
            t = mpool.tile([1, NZ], F32, tag="brow", bufs=3)
            nc.sync.dma_start(out=t, in_=b_d[name][:, :])
            return t

        def linear(x_tiles, name, nk, act=None, want32=False, want_bf=True):
            """col-layout linear: x (nk tiles [128,2] bf16) @ W[name] -> 8 tiles."""
            wts = load_w(name, nk)
            brow = load_brow(name)
            outs_bf, outs_32 = [], []
            for ot in range(NI):
                ps = ps_mm2.tile([128, BP], F32, tag="mm2")
                for kt in range(nk):
                    nc.tensor.matmul(ps, lhsT=wts(kt)[:, ot * 128:(ot + 1) * 128],
                                     rhs=x_tiles[kt],
                                     start=(kt == 0),
                                     stop=(kt == nk - 1 and brow is None))
                if brow is not None:
                    nc.tensor.matmul(ps, lhsT=brow[:, ot * 128:(ot + 1) * 128],
                                     rhs=ones2, start=False, stop=True)
                y32 = None
                if want32 or act == "lrelu":
                    y32 = apool.tile([128, BP], F32, tag="act32")
                    nc.vector.tensor_copy(out=y32, in_=ps)
                    if want32:
                        outs_32.append(y32)
                if want_bf:
                    y = apool.tile([128, BP], BF16, tag="act")
                    if act == "lrelu":
                        nc.vector.scalar_tensor_tensor(out=y, in0=y32, scalar=LRELU,
                                                       in1=y32, op0=ALU.mult, op1=ALU.max)
                    elif y32 is not None:
                        nc.vector.tensor_copy(out=y, in_=y32)
                    else:
                        nc.vector.tensor_copy(out=y, in_=ps)
                    outs_bf.append(y)
            return outs_bf, outs_32

        def layernorm(x32_tiles, want32=False):
            """LN over the 1024-dim (partitions x 8 tiles) of col-layout f32 tiles."""
            # sums and sums of squares via ones-matmul
            ps_s = ps_mm2.tile([1, BP], F32, tag="mm2")
            ps_q = ps_mm2.tile([1, BP], F32, tag="mm2")
            sq_tiles = []
            for kt in range(NI):
                sq = apool.tile([128, BP], F32, tag="sq", bufs=10)
                nc.vector.tensor_mul(sq, x32_tiles[kt], x32_tiles[kt])
                sq_tiles.append(sq)
            for kt in range(NI):
                nc.tensor.matmul(ps_s, lhsT=ones_col, rhs=x32_tiles[kt],
                                 start=(kt == 0), stop=(kt == NI - 1))
            for kt in range(NI):
                nc.tensor.matmul(ps_q, lhsT=ones_col, rhs=sq_tiles[kt],
                                 start=(kt == 0), stop=(kt == NI - 1))
            mr = mpool.tile([1, 2 * BP], F32, tag="mr")  # [m_b0, m_b1, r_b0, r_b1]
            m_ap = mr[:, 0:BP]
            nc.vector.tensor_scalar_mul(m_ap, ps_s, 1.0 / NZ)
            e2 = mpool.tile([1, BP], F32, tag="e2")
            nc.vector.tensor_scalar_mul(e2, ps_q, 1.0 / NZ)
            msq = mpool.tile([1, BP], F32, tag="msq")
            nc.vector.tensor_mul(msq, m_ap, m_ap)
            var = mpool.tile([1, BP], F32, tag="var")
            nc.vector.tensor_sub(var, e2, msq)
            sd = mpool.tile([1, BP], F32, tag="sd")
            nc.scalar.activation(out=sd, in_=var, func=AF.Sqrt, bias=eps_t, scale=1.0)
            nc.vector.reciprocal(mr[:, BP:2 * BP], sd)
            ps_bc = ps_mm2.tile([128, 2 * BP], F32, tag="mm2")
            nc.tensor.matmul(ps_bc, lhsT=ones_row, rhs=mr, start=True, stop=True)
            outs_bf, outs_32 = [], []
            for kt in range(NI):
                t0 = apool.tile([128, BP], F32, tag="lnt", bufs=10)
                nc.vector.tensor_sub(t0, x32_tiles[kt], ps_bc[:, 0:BP])
                y = apool.tile([128, BP], BF16, tag="act")
                nc.vector.tensor_mul(y, t0, ps_bc[:, BP:2 * BP])
                outs_bf.append(y)
                if want32:
                    y32 = apool.tile([128, BP], F32, tag="act32")
                    nc.vector.tensor_mul(y32, t0, ps_bc[:, BP:2 * BP])
                    outs_32.append(y32)
            return outs_bf, outs_32

        # ---- query_net -------------------------------------------------
        x = [qin_sb[:, kt, :] for kt in range(16)]
        x, _ = linear(x, "qn0", 16, act="lrelu")
        for i in range(1, 4):
            x, _ = linear(x, f"qn{i}", NI, act="lrelu")
        query_bf, _ = linear(x, "qn4", NI, act=None)
        query_32 = None  # only needed for residual; rebuilt per layer below
        # we need f32 query for residual adds: recompute as copies
        q32 = []
        for kt in range(NI):
            t = apool.tile([128, BP], F32, tag="act32")
            nc.vector.tensor_copy(out=t, in_=query_bf[kt])
            q32.append(t)
        query_32 = q32

        # input streams: emitted here so their DMA packets queue AFTER the
        # query_net weights (PE needs those first) but well before use
        for b in range(BP):
            load_keysT(b)
            load_values(b)

        raw_32 = None
        raw_bf = None
        pnorm = [None, None]

        for l in range(N_LAYERS):
            last = l == N_LAYERS - 1
            # q projection (col layout)
            q_col, _ = linear(query_bf, f"wq{l}", NI, act=None)

            # qk^T = blockdiag(q) @ WkT   [8, 1024] per batch
            wkT = load_w(f"wkT{l}", NI)
            qk_col = [[None] * NI for _ in range(BP)]
            qkT_sb = [None, None]
            for b in range(BP):
                # block-diag stationary: qblk[kt] [128, 8] bf16, col kt = q head kt
                qblk = []
                for kt in range(H):
                    qb = apool.tile([128, H], BF16, tag="qblk", bufs=20, name="qb")
                    nc.vector.memset(qb, 0.0)
                    nc.vector.tensor_copy(out=qb[:, kt:kt + 1], in_=q_col[kt][:, b:b + 1])
                    qblk.append(qb)
                qkT_sb[b] = mpool.tile([H, NZ], BF16, tag="qkT", bufs=2, name="qkT")
                for c in range(2):
                    ps = ps_wide.tile([H, 512], F32, tag="wide")
                    for kt in range(H):
                        nc.tensor.matmul(ps,
                                         lhsT=qblk[kt],
                                         rhs=wkT(kt)[:, c * 512:(c + 1) * 512],
                                         start=(kt == 0), stop=(kt == H - 1))
                    nc.vector.tensor_copy(out=qkT_sb[b][:, c * 512:(c + 1) * 512], in_=ps)
                for it in range(NI):
                    tp = ps_tp.tile([128, H], BF16, tag="tp")
                    nc.tensor.transpose(out=tp, in_=qkT_sb[b][:, it * 128:(it + 1) * 128],
                                        identity=id_b[0:H, 0:H])
                    qc = apool.tile([128, H], BF16, tag="qkcol", bufs=20)
                    nc.vector.tensor_copy(out=qc, in_=tp)
                    qk_col[b][it] = qc

            # scb: per-head constant from k-bias (usually zero -> skipped)
            scb_sb = [None, None]
            if bias_nz.get(f"bk{l}", False):
                bkcol = mpool.tile([128, H], F32, tag="bkcol")
                nc.sync.dma_start(out=bkcol, in_=b_d[f"bk{l}"][:, :])
                ones512 = singles.tile([1, 512], F32, tag="ones512")
                nc.vector.memset(ones512, 1.0)
                bk_bf = mpool.tile([128, H], BF16, tag="bkbf")
                nc.vector.tensor_copy(out=bk_bf, in_=bkcol)
                for b in range(BP):
                    ps = ps_mm2.tile([1, H], F32, tag="mm2")
                    for h in range(H):
                        nc.tensor.matmul(ps[:, h:h + 1],
                                         lhsT=q_col[h][:, b:b + 1],
                                         rhs=bk_bf[:, h:h + 1],
                                         start=True, stop=True)
                    s = mpool.tile([1, H], F32, tag="scb")
                    nc.vector.tensor_copy(out=s, in_=ps)
                    scb_sb[b] = s

            # scores -> softmax -> pnorm -> pT -> wvals, pipelined per batch
            wv_sb = []
            for b in range(BP):
                probs = ppool.tile([H, T], F32, tag="probs")
                for c in range(4):
                    ps = ps_wide.tile([H, 512], F32, tag="wide")
                    for it in range(NI):
                        nc.tensor.matmul(ps,
                                         lhsT=qk_col[b][it],
                                         rhs=keysT_sb[b][it][:, c * 512:(c + 1) * 512],
                                         start=(it == 0),
                                         stop=(it == NI - 1 and scb_sb[b] is None))
                    if scb_sb[b] is not None:
                        nc.tensor.matmul(ps, lhsT=scb_sb[b], rhs=ones512,
                                         start=False, stop=True)
                    if cfg["mask"]:
                        nc.vector.tensor_add(probs[:, c * 512:(c + 1) * 512], ps,
                                             mask_sb[b][:, c * 512:(c + 1) * 512])
                    else:
                        nc.vector.tensor_copy(out=probs[:, c * 512:(c + 1) * 512], in_=ps)
                mx = mpool.tile([H, 1], F32, tag="mx")
                nc.vector.reduce_max(mx, probs, axis=AXX.X)
                nb = mpool.tile([H, 1], F32, tag="nb")
                nc.vector.tensor_scalar_mul(nb, mx, -SCALE)
                ssum = mpool.tile([H, 1], F32, tag="ssum")
                nc.scalar.activation(out=probs, in_=probs, func=AF.Exp,
                                     bias=nb, scale=SCALE, accum_out=ssum)
                rec = mpool.tile([H, 1], F32, tag="rec")
                nc.vector.reciprocal(rec, ssum)
                nc.vector.tensor_scalar(out=probs, in0=probs, scalar1=rec,
                                        scalar2=None, op0=ALU.mult)
                pnorm[b] = probs

                if last:
                    # attw = head-mean of pnorm: emit early so it overlaps
                    for c in range(4):
                        ps = ps_wide.tile([1, 512], F32, tag="wide")
                        nc.tensor.matmul(ps, lhsT=sel8,
                                         rhs=pnorm[b][:, c * 512:(c + 1) * 512],
                                         start=True, stop=True)
                        awc = mpool.tile([1, 512], F32, tag="awc", bufs=2, name="awc")
                        nc.vector.tensor_copy(out=awc, in_=ps)
                        nc.sync.dma_start(out=attw_d[b:b + 1, c * 512:(c + 1) * 512],
                                          in_=awc)

                # pT_b tiles [128, 8] bf16 + wvals_b: emitted per-b so batch
                # b0's wvals matmuls overlap batch b1's softmax
                pT_b = []
                for tt in range(NT):
                    tp = ps_tp.tile([128, H], F32, tag="tp", name="tp_p")
                    nc.tensor.transpose(out=tp,
                                        in_=pnorm[b][:, tt * 128:(tt + 1) * 128],
                                        identity=id_f[0:H, 0:H])
                    t = mpool.tile([128, H], BF16, tag="pT", bufs=36, name="pT_t")
                    nc.vector.tensor_copy(out=t, in_=tp)
                    pT_b.append(t)
                wv_b = mpool.tile([H, NZ], BF16, tag="wv", bufs=2, name="wv_b")
                for c in range(2):
                    ps = ps_wide.tile([H, 512], F32, tag="wide")
                    for tt in range(NT):
                        nc.tensor.matmul(ps,
                                         lhsT=pT_b[tt],
                                         rhs=values_sb[b][tt][:, c * 512:(c + 1) * 512],
                                         start=(tt == 0), stop=(tt == NT - 1))
                    nc.vector.tensor_copy(out=wv_b[:, c * 512:(c + 1) * 512], in_=ps)
                wv_sb.append(wv_b)
            # wvT tiles [128, 16] bf16 (cols b*8+h)
            wvT = []
            for it in range(NI):
                tp = ps_tp.tile([128, 2 * H], BF16, tag="tp")
                for b in range(BP):
                    nc.tensor.transpose(out=tp[:, b * H:(b + 1) * H],
                                        in_=wv_sb[b][:, it * 128:(it + 1) * 128],
                                        identity=id_b[0:H, 0:H])
                t = mpool.tile([128, 2 * H], BF16, tag="wvT", bufs=10)
                nc.vector.tensor_copy(out=t, in_=tp)
                wvT.append(t)

            # vals_col[h] [128, 2] = sum_it Wv[it, hblock]^T @ wvT[it][:, (b*8+h)]
            wv_w = load_w(f"wv{l}", NI)
            bv_row = load_brow(f"wv{l}")
            vals_col = []
            for h in range(H):
                ps = ps_mm2.tile([128, BP], F32, tag="mm2")
                for it in range(NI):
                    nc.tensor.matmul(ps,
                                     lhsT=wv_w(it)[:, h * 128:(h + 1) * 128],
                                     rhs=wvT[it][:, h::H],
                                     start=(it == 0),
                                     stop=(it == NI - 1 and bv_row is None))
                if bv_row is not None:
                    nc.tensor.matmul(ps, lhsT=bv_row[:, h * 128:(h + 1) * 128],
                                     rhs=ones2, start=False, stop=True)
                v = apool.tile([128, BP], BF16, tag="act")
                nc.vector.tensor_copy(out=v, in_=ps)
                vals_col.append(v)

            # attn_out
            raw_bf, raw_32 = linear(vals_col, f"wao{l}", NI, act=None,
                                    want32=True, want_bf=last)

            # x = LN(raw); pred; query = LN(pred(x) + query)
            x_bf, _ = layernorm(raw_32)
            p = x_bf
            for i in range(3):
                p, _ = linear(p, f"pred{l}_{i}", NI, act="lrelu")
            _, p32 = linear(p, f"pred{l}_3", NI, act=None, want32=True, want_bf=False)
            qnew = []
            for kt in range(NI):
                t = apool.tile([128, BP], F32, tag="qnew", bufs=10)
                nc.vector.tensor_add(t, p32[kt], query_32[kt])
                qnew.append(t)
            query_bf, query_32 = layernorm(qnew, want32=True)

        # ---- out = raw @ Wout ----------------------------------------
        wts = load_w("wout", NI)
        brow = load_brow("wout")
        outcol = singles.tile([128, NI * BP], F32, tag="outcol")
        for ot in range(NI):
            ps = ps_mm2.tile([128, BP], F32, tag="mm2")
            for kt in range(NI):
                nc.tensor.matmul(ps, lhsT=wts(kt)[:, ot * 128:(ot + 1) * 128],
                                 rhs=raw_bf[kt], start=(kt == 0),
                                 stop=(kt == NI - 1 and brow is None))
            if brow is not None:
                nc.tensor.matmul(ps, lhsT=brow[:, ot * 128:(ot + 1) * 128],
                                 rhs=ones2, start=False, stop=True)
            nc.vector.tensor_copy(out=outcol[:, ot * BP:(ot + 1) * BP], in_=ps)
        outrow = singles.tile([BP, NZ], F32, tag="outrow")
        for ot in range(NI):
            tp = ps_tp.tile([BP, 128], F32, tag="tp", name="tp_out")
            nc.tensor.transpose(out=tp, in_=outcol[:, ot * BP:(ot + 1) * BP],
                                identity=id_f)
            nc.vector.tensor_copy(out=outrow[:, ot * 128:(ot + 1) * 128], in_=tp)
        nc.sync.dma_start(out=out_d[:, :], in_=outrow)

    nc.compile()
    return nc


def kernel(values, keys, query_input, start_ind, end_ind, params, _trace=False):
    in_maps, cfg = _host_prep(values, keys, query_input, start_ind, end_ind, params)
    nc = build(cfg)
    res = run_bass_kernel_spmd(nc, in_maps, core_ids=list(range(NCORES)), trace=_trace)
    LAST_RESULT["exec_time_ns"] = res.exec_time_ns
    LAST_RESULT["mean_exec_time_ns"] = res.mean_exec_time_ns
    LAST_RESULT["profile_json"] = res.profile_json
    out = np.concatenate([res.results[c]["out"] for c in range(NCORES)], axis=0)
    attw = np.concatenate([res.results[c]["attw"] for c in range(NCORES)], axis=0)
    return out.astype(np.float32), attw.astype(np.float32)


# revision 19
# speedup vs baseline: 1.1331x; 1.1331x over previous
"""Trainium2 Bass kernel for the 2-layer cross-attention module.

Sharding: data-parallel over batch B=16 -> 2 batch elements per core, 8 cores,
no collectives. Per core the algebra is restructured by linearity so the two
giant [T,NZ]@[NZ,NZ] projections disappear:

  scores[t,h] = sum_i keys[t,i] * qk[i,h],   qk[i,h] = sum_{d in head h} Wk[i,hd] q[hd]
  vals[hd]    = sum_i wvals[h,i] * Wv[i,hd], wvals[h,i] = sum_t p[t,h] values[t,i]

All matmul inputs are bf16 (f32 PSUM accumulation); softmax/layernorm/attw run
in f32. The small linear chain (query_net / pred / attn_out / out) runs in a
feature-on-partitions column layout [128, 2] per tile.
"""

import numpy as np
import ml_dtypes

import concourse.mybir as mybir
from concourse import bacc, tile
from concourse.bass_utils import run_bass_kernel_spmd
from concourse.masks import make_identity

F32 = mybir.dt.float32
BF16 = mybir.dt.bfloat16
ALU = mybir.AluOpType
AXX = mybir.AxisListType
AF = mybir.ActivationFunctionType

B, T, NZ, H, DK = 16, 2048, 1024, 8, 128
NCORES, BP = 8, 2
NT, NI = T // 128, NZ // 128  # 16, 8
SCALE = float(1.0 / np.sqrt(DK))  # * attention_temperature (1.0)
EPS = 1e-5
N_LAYERS = 2
LRELU = 0.2

LAST_RESULT = {}  # test harness introspection: exec_time_ns etc.


def _np(x):
    return np.asarray(x)


def _bf(x):
    return np.ascontiguousarray(np.asarray(x, dtype=np.float32).astype(ml_dtypes.bfloat16))


def _f32(x):
    return np.ascontiguousarray(np.asarray(x), dtype=np.float32)


def _host_prep(values, keys, query_input, start_ind, end_ind, params):
    """Marshal full inputs into per-core shard dicts + build config."""
    values = _np(values)
    keys = _np(keys)
    query_input = _np(query_input)
    start_ind = _f32(start_ind)
    end_ind = _f32(end_ind)

    # ---- weights: flatten into named f32/bf16 arrays -------------------
    # linear spec: name -> (W [din,dout] bf16, bias [dout] f32, nk)
    lin_w = {}
    lin_b = {}

    def add_lin(name, p):
        W, b = p
        W = _f32(W)
        lin_w[name] = _bf(W)
        lin_b[name] = _f32(b).reshape(1, -1)

    qn = params["query_net"]
    for i, p in enumerate(qn):
        add_lin(f"qn{i}", p)
    for l, lp in enumerate(params["layers"]):
        add_lin(f"wq{l}", lp["q"])
        add_lin(f"wv{l}", lp["v"])
        add_lin(f"wao{l}", lp["attn_out"])
        for i, p in enumerate(lp["pred"]):
            add_lin(f"pred{l}_{i}", p)
        # k projection: host-transpose W, bias handled via scb path
        Wk, bk = lp["k"]
        lin_w[f"wkT{l}"] = _bf(_f32(Wk).T)
        lin_b[f"wkT{l}"] = np.zeros((1, NZ), np.float32)  # unused
        lin_b[f"bk{l}"] = _f32(bk).reshape(1, -1)
    add_lin("wout", params["out"])

    bias_nonzero = {k: bool(np.any(v)) for k, v in lin_b.items()}

    # ---- mask ----------------------------------------------------------
    tt = np.arange(T, dtype=np.float32)
    mask = (tt[None, :] < np.floor(start_ind)[:, None]) | (
        tt[None, :] > np.ceil(end_ind)[:, None]
    )  # [B, T], True = masked out
    fully = mask.all(axis=1)
    maskadd = np.where(mask, np.float32(-1e30), np.float32(0.0)).astype(np.float32)
    maskadd[fully] = 0.0  # fully-masked row -> uniform softmax (matches reference)
    mask_nontrivial = bool(maskadd.any())

    cfg = {
        "bias_nonzero": bias_nonzero,
        "mask": mask_nontrivial,
    }

    # ---- per-core shards ----------------------------------------------
    in_maps = []
    for c in range(NCORES):
        sl = slice(BP * c, BP * (c + 1))
        m = {
            "keysT": _bf(keys[sl].transpose(0, 2, 1)),          # [BP, NZ, T]
            "values": _bf(values[sl]),                          # [BP, T, NZ]
            # col layout [128, 16, BP]: partition p, ktile k, batch b
            "qin": _bf(query_input[sl].T.reshape(16, 128, BP).transpose(1, 0, 2)),
        }
        for k, v in lin_w.items():
            m[k] = v
        for k, v in lin_b.items():
            if bias_nonzero[k]:
                if k.startswith("bk"):
                    # column pack [128, 8]: partition d, col h -> bk[h*128+d]
                    m[k + "_col"] = v.reshape(H, DK).T.copy()
                else:
                    m[k + "_bias"] = v
        if mask_nontrivial:
            # expand per (b, h, t): same row for all h
            m["maskadd"] = np.repeat(maskadd[sl][:, None, :], H, axis=1).copy()
        in_maps.append(m)
    return in_maps, cfg


def build(cfg):
    bias_nz = cfg["bias_nonzero"]
    nc = bacc.Bacc("TRN2", target_bir_lowering=False, debug=False)

    keysT_d = nc.dram_tensor("keysT", [BP, NZ, T], BF16, kind="ExternalInput")
    values_d = nc.dram_tensor("values", [BP, T, NZ], BF16, kind="ExternalInput")
    qin_d = nc.dram_tensor("qin", [128, 16, BP], BF16, kind="ExternalInput")

    w_d = {}
    b_d = {}

    def decl_lin(name, din):
        w_d[name] = nc.dram_tensor(name, [din, NZ], BF16, kind="ExternalInput")
        if bias_nz.get(name, False):
            b_d[name] = nc.dram_tensor(name + "_bias", [1, NZ], F32, kind="ExternalInput")

    decl_lin("qn0", 2 * NZ)
    for i in range(1, 5):
        decl_lin(f"qn{i}", NZ)
    for l in range(N_LAYERS):
        decl_lin(f"wq{l}", NZ)
        decl_lin(f"wv{l}", NZ)
        decl_lin(f"wao{l}", NZ)
        for i in range(4):
            decl_lin(f"pred{l}_{i}", NZ)
        w_d[f"wkT{l}"] = nc.dram_tensor(f"wkT{l}", [NZ, NZ], BF16, kind="ExternalInput")
        if bias_nz.get(f"bk{l}", False):
            b_d[f"bk{l}"] = nc.dram_tensor(f"bk{l}_col", [128, H], F32, kind="ExternalInput")
    decl_lin("wout", NZ)
    if cfg["mask"]:
        mask_d = nc.dram_tensor("maskadd", [BP, H, T], F32, kind="ExternalInput")

    out_d = nc.dram_tensor("out", [BP, NZ], F32, kind="ExternalOutput")
    attw_d = nc.dram_tensor("attw", [BP, T], F32, kind="ExternalOutput")

    from contextlib import ExitStack

    with tile.TileContext(nc) as tc, ExitStack() as ctx:
        singles = ctx.enter_context(tc.tile_pool(name="singles", bufs=1))
        wpool = ctx.enter_context(tc.tile_pool(name="wpool", bufs=2))
        apool = ctx.enter_context(tc.tile_pool(name="apool", bufs=64))
        ppool = ctx.enter_context(tc.tile_pool(name="ppool", bufs=2))
        mpool = ctx.enter_context(tc.tile_pool(name="mpool", bufs=3))
        ps_mm2 = ctx.enter_context(tc.tile_pool(name="ps_mm2", bufs=3, space="PSUM"))
        ps_wide = ctx.enter_context(tc.tile_pool(name="ps_wide", bufs=2, space="PSUM"))
        ps_tp = ctx.enter_context(tc.tile_pool(name="ps_tp", bufs=2, space="PSUM"))

        # ---- constants -------------------------------------------------
        id_f = singles.tile([128, 128], F32, tag="id_f")
        make_identity(nc, id_f)
        id_b = singles.tile([128, 128], BF16, tag="id_b")
        make_identity(nc, id_b)
        ones_col = singles.tile([128, 1], F32, tag="ones_col")
        nc.vector.memset(ones_col, 1.0)
        ones_row = singles.tile([1, 128], F32, tag="ones_row")
        nc.vector.memset(ones_row, 1.0)
        ones2 = singles.tile([1, 2], F32, tag="ones2")
        nc.vector.memset(ones2, 1.0)
        sel8 = singles.tile([8, 1], BF16, tag="sel8")
        nc.vector.memset(sel8, 1.0 / H)
        eps_t = singles.tile([1, 1], F32, tag="eps")
        nc.vector.memset(eps_t, EPS)

        # ---- persistent inputs (DMAs deferred: weights go first) -------
        # 3-D tiles: keysT_sb3[b] [128, NI, T]; values_sb3[b] [128, NT, NZ]
        keysT_sb3 = [singles.tile([128, NI, T], BF16, tag=f"kT{b}", name=f"kT{b}")
                     for b in range(BP)]
        values_sb3 = [singles.tile([128, NT, NZ], BF16, tag=f"v{b}", name=f"v{b}")
                      for b in range(BP)]
        keysT_sb = [[keysT_sb3[b][:, it, :] for it in range(NI)] for b in range(BP)]
        values_sb = [[values_sb3[b][:, tt, :] for tt in range(NT)] for b in range(BP)]

        def load_keysT(b):
            nc.sync.dma_start(out=keysT_sb3[b],
                              in_=keysT_d[b].rearrange("(it p) t -> p it t", p=128))

        def load_values(b):
            nc.sync.dma_start(out=values_sb3[b],
                              in_=values_d[b].rearrange("(tt p) i -> p tt i", p=128))

        qin_sb = singles.tile([128, 16, BP], BF16, tag="qin")
        nc.sync.dma_start(out=qin_sb, in_=qin_d[:, :, :])
        if cfg["mask"]:
            mask_sb = [singles.tile([H, T], F32, tag=f"mask{b}", name=f"mask{b}") for b in range(BP)]
            for b in range(BP):
                nc.sync.dma_start(out=mask_sb[b], in_=mask_d[b, :, :])

        # ---- helpers ---------------------------------------------------
        def load_w(name, nk):
            """Returns a lambda kt -> [128, 1024] AP; one batched DMA per 8 ktiles."""
            segs = []
            for s0 in range(0, nk, 8):
                w = wpool.tile([128, 8, NZ], BF16, tag="w", name=f"w_{name}_{s0}")
                nc.sync.dma_start(
                    out=w,
                    in_=w_d[name][s0 * 128:(s0 + 8) * 128, :].rearrange(
                        "(k p) n -> p k n", p=128))
                segs.append(w)
            return lambda kt: segs[kt // 8][:, kt % 8, :]

        def load_brow(name):
            if not bias_nz.get(name, False):
                return None
            t = mpool.tile([1, NZ], F32, tag="brow", bufs=3)
            nc.sync.dma_start(out=t, in_=b_d[name][:, :])
            return t

        def linear(x_tiles, name, nk, act=None, want32=False, want_bf=True):
            """col-layout linear: x (nk tiles [128,2] bf16) @ W[name] -> 8 tiles."""
            wts = load_w(name, nk)
            brow = load_brow(name)
            outs_bf, outs_32 = [], []
            for ot in range(NI):
                ps = ps_mm2.tile([128, BP], F32, tag="mm2")
                for kt in range(nk):
                    nc.tensor.matmul(ps, lhsT=wts(kt)[:, ot * 128:(ot + 1) * 128],
                                     rhs=x_tiles[kt],
                                     start=(kt == 0),
                                     stop=(kt == nk - 1 and brow is None))
                if brow is not None:
                    nc.tensor.matmul(ps, lhsT=brow[:, ot * 128:(ot + 1) * 128],
                                     rhs=ones2, start=False, stop=True)
                y32 = None
                if want32 or act == "lrelu":
                    y32 = apool.tile([128, BP], F32, tag="act32")
                    nc.vector.tensor_copy(out=y32, in_=ps)
                    if want32:
                        outs_32.append(y32)
                if want_bf:
                    y = apool.tile([128, BP], BF16, tag="act")
                    if act == "lrelu":
                        nc.vector.scalar_tensor_tensor(out=y, in0=y32, scalar=LRELU,
                                                       in1=y32, op0=ALU.mult, op1=ALU.max)
                    elif y32 is not None:
                        nc.vector.tensor_copy(out=y, in_=y32)
                    else:
                        nc.vector.tensor_copy(out=y, in_=ps)
                    outs_bf.append(y)
            return outs_bf, outs_32

        def layernorm(x32_tiles, want32=False):
            """LN over the 1024-dim (partitions x 8 tiles) of col-layout f32 tiles."""
            # sums and sums of squares via ones-matmul
            ps_s = ps_mm2.tile([1, BP], F32, tag="mm2")
            ps_q = ps_mm2.tile([1, BP], F32, tag="mm2")
            sq_tiles = []
            for kt in range(NI):
                sq = apool.tile([128, BP], F32, tag="sq", bufs=10)
                nc.vector.tensor_mul(sq, x32_tiles[kt], x32_tiles[kt])
                sq_tiles.append(sq)
            for kt in range(NI):
                nc.tensor.matmul(ps_s, lhsT=ones_col, rhs=x32_tiles[kt],
                                 start=(kt == 0), stop=(kt == NI - 1))
            for kt in range(NI):
                nc.tensor.matmul(ps_q, lhsT=ones_col, rhs=sq_tiles[kt],
                                 start=(kt == 0), stop=(kt == NI - 1))
            mr = mpool.tile([1, 2 * BP], F32, tag="mr")  # [m_b0, m_b1, r_b0, r_b1]
            m_ap = mr[:, 0:BP]
            nc.vector.tensor_scalar_mul(m_ap, ps_s, 1.0 / NZ)
            e2 = mpool.tile([1, BP], F32, tag="e2")
            nc.vector.tensor_scalar_mul(e2, ps_q, 1.0 / NZ)
            msq = mpool.tile([1, BP], F32, tag="msq")
            nc.vector.tensor_mul(msq, m_ap, m_ap)
            var = mpool.tile([1, BP], F32, tag="var")
            nc.vector.tensor_sub(var, e2, msq)
            sd = mpool.tile([1, BP], F32, tag="sd")
            nc.scalar.activation(out=sd, in_=var, func=AF.Sqrt, bias=eps_t, scale=1.0)
            nc.vector.reciprocal(mr[:, BP:2 * BP], sd)
            ps_bc = ps_mm2.tile([128, 2 * BP], F32, tag="mm2")
            nc.tensor.matmul(ps_bc, lhsT=ones_row, rhs=mr, start=True, stop=True)
            outs_bf, outs_32 = [], []
            for kt in range(NI):
                t0 = apool.tile([128, BP], F32, tag="lnt", bufs=10)
                nc.vector.tensor_sub(t0, x32_tiles[kt], ps_bc[:, 0:BP])
                y = apool.tile([128, BP], BF16, tag="act")
                nc.vector.tensor_mul(y, t0, ps_bc[:, BP:2 * BP])
                outs_bf.append(y)
                if want32:
                    y32 = apool.tile([128, BP], F32, tag="act32")
                    nc.vector.tensor_mul(y32, t0, ps_bc[:, BP:2 * BP])
                    outs_32.append(y32)
            return outs_bf, outs_32

        # ---- query_net -------------------------------------------------
        x = [qin_sb[:, kt, :] for kt in range(16)]
        x, _ = linear(x, "qn0", 16, act="lrelu")
        for i in range(1, 4):
            x, _ = linear(x, f"qn{i}", NI, act="lrelu")
        query_bf, _ = linear(x, "qn4", NI, act=None)
        query_32 = None  # only needed for residual; rebuilt per layer below
        # we need f32 query for residual adds: recompute as copies
        q32 = []
        for kt in range(NI):
            t = apool.tile([128, BP], F32, tag="act32")
            nc.vector.tensor_copy(out=t, in_=query_bf[kt])
            q32.append(t)
        query_32 = q32

        raw_32 = None
        raw_bf = None
        pnorm = [None, None]

        for l in range(N_LAYERS):
            last = l == N_LAYERS - 1
            # q projection (col layout)
            q_col, _ = linear(query_bf, f"wq{l}", NI, act=None)

            # qk^T = blockdiag(q) @ WkT   [8, 1024] per batch
            wkT = load_w(f"wkT{l}", NI)
            if l == 0:
                load_keysT(0)
                load_keysT(1)
            qk_col = [[None] * NI for _ in range(BP)]
            qkT_sb = [None, None]
            for b in range(BP):
                # block-diag stationary: qblk[kt] [128, 8] bf16, col kt = q head kt
                qblk = []
                for kt in range(H):
                    qb = apool.tile([128, H], BF16, tag="qblk", bufs=20, name="qb")
                    nc.vector.memset(qb, 0.0)
                    nc.vector.tensor_copy(out=qb[:, kt:kt + 1], in_=q_col[kt][:, b:b + 1])
                    qblk.append(qb)
                qkT_sb[b] = mpool.tile([H, NZ], BF16, tag="qkT", bufs=2, name="qkT")
                for c in range(2):
                    ps = ps_wide.tile([H, 512], F32, tag="wide")
                    for kt in range(H):
                        nc.tensor.matmul(ps,
                                         lhsT=qblk[kt],
                                         rhs=wkT(kt)[:, c * 512:(c + 1) * 512],
                                         start=(kt == 0), stop=(kt == H - 1))
                    nc.vector.tensor_copy(out=qkT_sb[b][:, c * 512:(c + 1) * 512], in_=ps)
                for it in range(NI):
                    tp = ps_tp.tile([128, H], BF16, tag="tp")
                    nc.tensor.transpose(out=tp, in_=qkT_sb[b][:, it * 128:(it + 1) * 128],
                                        identity=id_b[0:H, 0:H])
                    qc = apool.tile([128, H], BF16, tag="qkcol", bufs=20)
                    nc.vector.tensor_copy(out=qc, in_=tp)
                    qk_col[b][it] = qc

            # scb: per-head constant from k-bias (usually zero -> skipped)
            scb_sb = [None, None]
            if bias_nz.get(f"bk{l}", False):
                bkcol = mpool.tile([128, H], F32, tag="bkcol")
                nc.sync.dma_start(out=bkcol, in_=b_d[f"bk{l}"][:, :])
                ones512 = singles.tile([1, 512], F32, tag="ones512")
                nc.vector.memset(ones512, 1.0)
                bk_bf = mpool.tile([128, H], BF16, tag="bkbf")
                nc.vector.tensor_copy(out=bk_bf, in_=bkcol)
                for b in range(BP):
                    ps = ps_mm2.tile([1, H], F32, tag="mm2")
                    for h in range(H):
                        nc.tensor.matmul(ps[:, h:h + 1],
                                         lhsT=q_col[h][:, b:b + 1],
                                         rhs=bk_bf[:, h:h + 1],
                                         start=True, stop=True)
                    s = mpool.tile([1, H], F32, tag="scb")
                    nc.vector.tensor_copy(out=s, in_=ps)
                    scb_sb[b] = s

            if l == 0:
                load_values(0)
                load_values(1)

            # scores -> softmax -> pnorm -> pT -> wvals, pipelined per batch
            wv_sb = []
            for b in range(BP):
                probs = ppool.tile([H, T], BF16, tag="probs")
                for c in range(4):
                    ps = ps_wide.tile([H, 512], F32, tag="wide")
                    for it in range(NI):
                        nc.tensor.matmul(ps,
                                         lhsT=qk_col[b][it],
                                         rhs=keysT_sb[b][it][:, c * 512:(c + 1) * 512],
                                         start=(it == 0),
                                         stop=(it == NI - 1 and scb_sb[b] is None))
                    if scb_sb[b] is not None:
                        nc.tensor.matmul(ps, lhsT=scb_sb[b], rhs=ones512,
                                         start=False, stop=True)
                    if cfg["mask"]:
                        nc.vector.tensor_add(probs[:, c * 512:(c + 1) * 512], ps,
                                             mask_sb[b][:, c * 512:(c + 1) * 512])
                    else:
                        nc.scalar.copy(out=probs[:, c * 512:(c + 1) * 512], in_=ps)
                mx = mpool.tile([H, 1], F32, tag="mx")
                nc.vector.reduce_max(mx, probs, axis=AXX.X)
                nb = mpool.tile([H, 1], F32, tag="nb")
                nc.vector.tensor_scalar_mul(nb, mx, -SCALE)
                ssum = mpool.tile([H, 1], F32, tag="ssum")
                nc.scalar.activation(out=probs, in_=probs, func=AF.Exp,
                                     bias=nb, scale=SCALE, accum_out=ssum)
                rec = mpool.tile([H, 1], F32, tag="rec")
                nc.vector.reciprocal(rec, ssum)
                nc.vector.tensor_scalar(out=probs, in0=probs, scalar1=rec,
                                        scalar2=None, op0=ALU.mult)
                pnorm[b] = probs

                if last:
                    # attw = head-mean of pnorm: emit early so it overlaps
                    for c in range(4):
                        ps = ps_wide.tile([1, 512], F32, tag="wide")
                        nc.tensor.matmul(ps, lhsT=sel8,
                                         rhs=pnorm[b][:, c * 512:(c + 1) * 512],
                                         start=True, stop=True)
                        awc = mpool.tile([1, 512], F32, tag="awc", bufs=2, name="awc")
                        nc.vector.tensor_copy(out=awc, in_=ps)
                        nc.sync.dma_start(out=attw_d[b:b + 1, c * 512:(c + 1) * 512],
                                          in_=awc)

                # pT_b tiles [128, 8] bf16 + wvals_b: emitted per-b so batch
                # b0's wvals matmuls overlap batch b1's softmax
                pT_b = []
                for tt in range(NT):
                    tp = ps_tp.tile([128, H], BF16, tag="tp", name="tp_p")
                    nc.tensor.transpose(out=tp,
                                        in_=pnorm[b][:, tt * 128:(tt + 1) * 128],
                                        identity=id_b[0:H, 0:H])
                    t = mpool.tile([128, H], BF16, tag="pT", bufs=36, name="pT_t")
                    nc.vector.tensor_copy(out=t, in_=tp)
                    pT_b.append(t)
                wv_b = mpool.tile([H, NZ], BF16, tag="wv", bufs=2, name="wv_b")
                for c in range(2):
                    ps = ps_wide.tile([H, 512], F32, tag="wide")
                    for tt in range(NT):
                        nc.tensor.matmul(ps,
                                         lhsT=pT_b[tt],
                                         rhs=values_sb[b][tt][:, c * 512:(c + 1) * 512],
                                         start=(tt == 0), stop=(tt == NT - 1))
                    nc.vector.tensor_copy(out=wv_b[:, c * 512:(c + 1) * 512], in_=ps)
                wv_sb.append(wv_b)
            # wvT tiles [128, 16] bf16 (cols b*8+h)
            wvT = []
            for it in range(NI):
                tp = ps_tp.tile([128, 2 * H], BF16, tag="tp")
                for b in range(BP):
                    nc.tensor.transpose(out=tp[:, b * H:(b + 1) * H],
                                        in_=wv_sb[b][:, it * 128:(it + 1) * 128],
                                        identity=id_b[0:H, 0:H])
                t = mpool.tile([128, 2 * H], BF16, tag="wvT", bufs=10)
                nc.vector.tensor_copy(out=t, in_=tp)
                wvT.append(t)

            # vals_col[h] [128, 2] = sum_it Wv[it, hblock]^T @ wvT[it][:, (b*8+h)]
            wv_w = load_w(f"wv{l}", NI)
            bv_row = load_brow(f"wv{l}")
            vals_col = []
            for h in range(H):
                ps = ps_mm2.tile([128, BP], F32, tag="mm2")
                for it in range(NI):
                    nc.tensor.matmul(ps,
                                     lhsT=wv_w(it)[:, h * 128:(h + 1) * 128],
                                     rhs=wvT[it][:, h::H],
                                     start=(it == 0),
                                     stop=(it == NI - 1 and bv_row is None))
                if bv_row is not None:
                    nc.tensor.matmul(ps, lhsT=bv_row[:, h * 128:(h + 1) * 128],
                                     rhs=ones2, start=False, stop=True)
                v = apool.tile([128, BP], BF16, tag="act")
                nc.vector.tensor_copy(out=v, in_=ps)
                vals_col.append(v)

            # attn_out
            raw_bf, raw_32 = linear(vals_col, f"wao{l}", NI, act=None,
                                    want32=True, want_bf=last)

            # x = LN(raw); pred; query = LN(pred(x) + query)
            x_bf, _ = layernorm(raw_32)
            p = x_bf
            for i in range(3):
                p, _ = linear(p, f"pred{l}_{i}", NI, act="lrelu")
            _, p32 = linear(p, f"pred{l}_3", NI, act=None, want32=True, want_bf=False)
            qnew = []
            for kt in range(NI):
                t = apool.tile([128, BP], F32, tag="qnew", bufs=10)
                nc.vector.tensor_add(t, p32[kt], query_32[kt])
                qnew.append(t)
            query_bf, query_32 = layernorm(qnew, want32=True)

        # ---- out = raw @ Wout ----------------------------------------
        wts = load_w("wout", NI)
        brow = load_brow("wout")
        outcol = singles.tile([128, NI * BP], F32, tag="outcol")
        for ot in range(NI):
            ps = ps_mm2.tile([128, BP], F32, tag="mm2")
            for kt in range(NI):
                nc.tensor.matmul(ps, lhsT=wts(kt)[:, ot * 128:(ot + 1) * 128],
                                 rhs=raw_bf[kt], start=(kt == 0),
                                 stop=(kt == NI - 1 and brow is None))
            if brow is not None:
                nc.tensor.matmul(ps, lhsT=brow[:, ot * 128:(ot + 1) * 128],
                                 rhs=ones2, start=False, stop=True)
            nc.vector.tensor_copy(out=outcol[:, ot * BP:(ot + 1) * BP], in_=ps)
        outrow = singles.tile([BP, NZ], F32, tag="outrow")
        for ot in range(NI):
            tp = ps_tp.tile([BP, 128], F32, tag="tp", name="tp_out")
            nc.tensor.transpose(out=tp, in_=outcol[:, ot * BP:(ot + 1) * BP],
                                identity=id_f)
            nc.vector.tensor_copy(out=outrow[:, ot * 128:(ot + 1) * 128], in_=tp)
        nc.sync.dma_start(out=out_d[:, :], in_=outrow)

    nc.compile()
    return nc


def kernel(values, keys, query_input, start_ind, end_ind, params, _trace=False):
    in_maps, cfg = _host_prep(values, keys, query_input, start_ind, end_ind, params)
    nc = build(cfg)
    res = run_bass_kernel_spmd(nc, in_maps, core_ids=list(range(NCORES)), trace=_trace)
    LAST_RESULT["exec_time_ns"] = res.exec_time_ns
    LAST_RESULT["mean_exec_time_ns"] = res.mean_exec_time_ns
    LAST_RESULT["profile_json"] = res.profile_json
    out = np.concatenate([res.results[c]["out"] for c in range(NCORES)], axis=0)
    attw = np.concatenate([res.results[c]["attw"] for c in range(NCORES)], axis=0)
    return out.astype(np.float32), attw.astype(np.float32)
